# revision 23
# baseline (speedup 1.0000x reference)
"""Single-head attention (B=8, S=2048, d_model=dk=dv=1024) on 8 TRN2 NeuronCores.

Strategy: data-parallel over batch, one batch element per core, SPMD.

Algebraic reduction vs the naive formulation: since softmax over s is
invariant to per-row (q) offsets, scores = (x Wq + bq)(x Wk + bk)^T reduces to
scores_eff[i,j] = x_i (Wq Wk^T) x_j^T + x_j (Wk bq), dropping the k-projection
entirely. M = Wq Wk^T and c = Wk bq are precomputed on host; the w = x c term
enters as a per-partition bias in the exp activation. This removes one of the
three [S,D]x[D,D] projections.

fp8 acceleration (flag-controlled): the two S x S matmuls (scores, AV) run in
fp8e4m3 with DoubleRow perf mode (2 k-groups per instruction, 2x PE
throughput). For the AV matmul, probs are stored as d8 = fp8(exp(z) - K) for
a hardcoded constant K ~ E[exp(z)]: the quantization noise then scales with
the small deviation rather than the full magnitude. The exact correction
out = (d8 @ v8 + K * colsum(v)) / denom uses colsum(v) computed on host in
f32, replicated across partitions; denom = colsum(d8) + S*K comes from
ones-stationary DoubleRow matmuls plus a DRAM-bounce transpose.

Matmul accumulation is always fp32 in PSUM. Phase 1 (projections t = x M,
v = x Wv, w = x c) stays bf16.
"""

import os
import sys

import numpy as np

try:
    import concourse.bass as bass  # noqa: F401
except ImportError:
    sys.path.insert(0, "/opt/trn_rl_repo")

import ml_dtypes

import concourse.bass as bass
import concourse.tile as tile
from concourse import bacc, mybir
from concourse import bass_utils

BF16 = mybir.dt.bfloat16
F32 = mybir.dt.float32
FP8 = mybir.dt.float8e4

B = 8
S = 2048
D = 1024  # d_model
DK = 1024
DV = 1024
P = 128  # partitions
NT = 512  # matmul free-dim tile (one PSUM bank of fp32)

D_T = D // P      # 8   contraction tiles over d_model
DK_T = DK // P    # 8   partition tiles of tT
S_T = S // P      # 16  partition tiles of v / probsT / out
S_N = S // NT     # 4   free-dim chunks over S
DV_N = DV // NT   # 2   free-dim chunks over dv
D_PAIR = D_T // 2   # 4  fp8 DoubleRow pairs over d
S_PAIR = S_T // 2   # 8  fp8 DoubleRow pairs over s

SCALE = 1.0 / float(np.sqrt(np.float32(DK)))
K_EZ = 1.078  # ~E[exp(scale*scores)] for this input distribution

P2_FP8 = bool(int(os.environ.get("KERNEL_P2_FP8", "1")))
P3_FP8 = bool(int(os.environ.get("KERNEL_P3_FP8", "1")))

DoubleRow = mybir.MatmulPerfMode.DoubleRow
Exp = mybir.ActivationFunctionType.Exp
Copy = mybir.ActivationFunctionType.Copy


def _emit(nc, p2_fp8, p3_fp8):
    xT_d = nc.dram_tensor("xT", [D, S], BF16, kind="ExternalInput").ap()
    Wm_d = nc.dram_tensor("Wm", [D, DK], BF16, kind="ExternalInput").ap()
    Wv_d = nc.dram_tensor("Wv", [D, DV], BF16, kind="ExternalInput").ap()
    wcol_d = nc.dram_tensor("wcol", [P, S_T], F32, kind="ExternalInput").ap()
    biasv_d = nc.dram_tensor("biasv", [P, DV], F32, kind="ExternalInput").ap()
    if p3_fp8:
        kvs_d = nc.dram_tensor("kvsum", [P, DV], F32, kind="ExternalInput").ap()
    out_d = nc.dram_tensor("out", [S, DV], F32, kind="ExternalOutput").ap()

    with tile.TileContext(nc) as tc:
        with (
            tc.tile_pool(name="persist", bufs=1) as persist,
            tc.tile_pool(name="dscr", bufs=1, space="DRAM") as dscr,
        ):
            # x^T stays resident: phase 1 contracts over it, phase 2 uses it
            # (or its fp8 copy) as the stationary side of the scores matmul.
            xTs = persist.tile([P, D_T * S], BF16, name="xTs", tag="xTs")
            if p2_fp8:
                xP = [persist.tile([P, 2 * S], FP8, name=f"xP{c}", tag=f"xP{c}")
                      for c in range(D_PAIR)]
                tP = [persist.tile([P, 2 * S], FP8, name=f"tP{c}", tag=f"tP{c}")
                      for c in range(D_PAIR)]
                tT = None
            else:
                xP = tP = None
                tT = [persist.tile([P, S], BF16, name=f"tT{i}", tag=f"tT{i}")
                      for i in range(DK_T)]
            if p3_fp8:
                v8 = [persist.tile([P, 2 * DV], FP8, name=f"v8{c}", tag=f"v8{c}")
                      for c in range(S_PAIR)]
                v = None
                kvs = persist.tile([P, DV], F32, name="kvs", tag="kvs")
                # [128, 2, 128] all-ones stationary for the denominator
                # colsums (1-col-per-group stationaries fail the dual-fp8
                # Ldweights ISA check, so the colsum output is replicated
                # across all 128 partitions instead).
                ones8 = persist.tile([P, 2 * P], FP8, name="ones8", tag="ones8")
                nc.vector.memset(ones8, 1.0)
            else:
                v8 = kvs = ones8 = None
                v = [persist.tile([P, DV], BF16, name=f"v{i}", tag=f"v{i}")
                     for i in range(S_T)]
                ones = persist.tile([P, 1], BF16, name="ones", tag="ones")
                nc.vector.memset(ones, 1.0)
            wcol = persist.tile([P, S_T], F32, name="wcol", tag="wcol")
            recip = persist.tile([P, S_T], F32, name="recip", tag="recip")

            _phase1(nc, tc, xT_d, Wm_d, Wv_d, wcol_d, biasv_d,
                    kvs_d if p3_fp8 else None, xTs, xP, tP, tT, v8, v, kvs,
                    wcol, p2_fp8, p3_fp8)

            with tc.tile_pool(name="probs", bufs=1) as probs_pool:
                if p3_fp8:
                    dP = [probs_pool.tile([P, 2 * S], FP8, name=f"dP{c}",
                                          tag=f"dP{c}") for c in range(S_PAIR)]
                    probsT = None
                else:
                    dP = None
                    probsT = [probs_pool.tile([P, S], BF16, name=f"pT{i}",
                                              tag=f"pT{i}") for i in range(S_T)]
                _phase2(nc, tc, dscr, xTs, xP, tP, tT, dP, probsT,
                        ones8 if p3_fp8 else ones, wcol, recip, p2_fp8, p3_fp8)
                _phase3(nc, tc, dP, probsT, v8, v, kvs, recip, out_d, p3_fp8)


def _phase1(nc, tc, xT_d, Wm_d, Wv_d, wcol_d, biasv_d, kvs_d, xTs, xP,
            tP, tT, v8, v, kvs, wcol, p2_fp8, p3_fp8):
    """tT = M^T x^T, v = x Wv + bv (w comes precomputed from host)."""
    with tc.tile_pool(name="inp", bufs=1) as inp:
        Wms = inp.tile([P, D_T * DK], BF16, name="Wms", tag="Wms")
        Wvs = inp.tile([P, D_T * DV], BF16, name="Wvs", tag="Wvs")
        biasv = inp.tile([P, DV], F32, name="biasv", tag="biasv")

        # Warmup matmuls on memset tiles: the PE pstate ramp (0.65 -> 1.2 ->
        # 2.4 GHz over ~3us of continuous execution) burns off during the
        # DMA lead-in instead of inflating the first real chains.
        wst = inp.tile([P, P], BF16, name="wst", tag="wst")
        wmv = inp.tile([P, NT], BF16, name="wmv", tag="wmv")
        nc.vector.memset(wst, 0.0)
        nc.vector.memset(wmv, 0.0)
        with tc.tile_pool(name="pwarm", bufs=2, space="PSUM") as pwarm:
            for i in range(14):
                wps = pwarm.tile([P, NT], F32, name="wps", tag="wps", bufs=2)
                nc.tensor.matmul(wps, wst, wmv, start=True, stop=True)

        xT3 = xTs.rearrange("p (c s) -> p c s", c=D_T)
        Wm3 = Wms.rearrange("p (c k) -> p c k", c=D_T)
        xTd3 = xT_d.rearrange("(c p) s -> p c s", p=P)
        Wmd3 = Wm_d.rearrange("(c p) k -> p c k", p=P)
        Wvd3 = Wv_d.rearrange("(c p) k -> p c k", p=P)

        # DMA order = consumption order. The first accumulation chain needs
        # Wm's m=0 block plus xT's n=0 chunk (kc-ascending), so issue those
        # first at fine granularity to cut the lead-in stall.
        nc.sync.dma_start(out=Wm3[:, :, 0:P], in_=Wmd3[:, :, 0:P])
        nc.scalar.dma_start(out=xT3[:, :, 0:NT], in_=xTd3[:, :, 0:NT])
        for m in range(1, DK_T):
            nc.sync.dma_start(
                out=Wm3[:, :, m * P:(m + 1) * P], in_=Wmd3[:, :, m * P:(m + 1) * P]
            )
        for n in range(1, S_N):
            nc.scalar.dma_start(
                out=xT3[:, :, n * NT:(n + 1) * NT], in_=xTd3[:, :, n * NT:(n + 1) * NT]
            )
        nc.sync.dma_start(out=Wvs, in_=Wvd3)
        nc.gpsimd.dma_start(out=wcol, in_=wcol_d)
        nc.gpsimd.dma_start(out=biasv, in_=biasv_d)
        if kvs_d is not None:
            nc.gpsimd.dma_start(out=kvs, in_=kvs_d)

        def Wm_sl(kc, m):
            return Wms[:, kc * DK + m * P: kc * DK + (m + 1) * P]

        def xT_sl(kc, lo, hi):
            return xTs[:, kc * S + lo: kc * S + hi]

        with tc.tile_pool(name="ps1t", bufs=4, space="PSUM") as ps1t, \
             tc.tile_pool(name="ps1v", bufs=4, space="PSUM") as ps1v:
            # tT[m*P+p, i] = sum_d M[d, m*P+p] * xT[d, i]
            for n in range(S_N):
                # fp8 copy of x^T for the DoubleRow scores matmul, emitted
                # before the chain-gated last-n copies so the DVE does them
                # while waiting on the n=3 chains rather than at phase-2 start
                if p2_fp8 and n == S_N - 1:
                    for kc in range(D_T):
                        with nc.allow_low_precision("fp8 scores by design"):
                            nc.vector.tensor_copy(
                                xP[kc // 2][:, (kc % 2) * S:(kc % 2 + 1) * S],
                                xT_sl(kc, 0, S),
                            )
                for m in range(DK_T):
                    ps = ps1t.tile([P, NT], F32, name="ps_t", tag="ps1t", bufs=4)
                    for kc in range(D_T):
                        nc.tensor.matmul(
                            ps,
                            Wm_sl(kc, m),
                            xT_sl(kc, n * NT, (n + 1) * NT),
                            start=(kc == 0),
                            stop=(kc == D_T - 1),
                        )
                    if p2_fp8:
                        dst = tP[m // 2][:, (m % 2) * S + n * NT:
                                         (m % 2) * S + (n + 1) * NT]
                        with nc.allow_low_precision("fp8 scores by design"):
                            nc.vector.tensor_copy(dst, ps)
                    else:
                        nc.vector.tensor_copy(tT[m][:, n * NT:(n + 1) * NT], ps)

            # v[m*P+p, j] = sum_d xT[d, m*P+p] * Wv[d, j] + bv
            for m in range(S_T):
                for nv in range(DV_N):
                    ps = ps1v.tile([P, NT], F32, name="ps_v", tag="ps1v", bufs=4)
                    for kc in range(D_T):
                        nc.tensor.matmul(
                            ps,
                            xT_sl(kc, m * P, (m + 1) * P),
                            Wvs[:, kc * DV + nv * NT: kc * DV + (nv + 1) * NT],
                            start=(kc == 0),
                            stop=(kc == D_T - 1),
                        )
                    bsl = biasv[:, nv * NT:(nv + 1) * NT]
                    if p3_fp8:
                        dst = v8[m // 2][:, (m % 2) * DV + nv * NT:
                                         (m % 2) * DV + (nv + 1) * NT]
                        with nc.allow_low_precision("fp8 AV by design"):
                            nc.vector.tensor_add(dst, ps, bsl)
                    else:
                        nc.vector.tensor_add(v[m][:, nv * NT:(nv + 1) * NT], ps, bsl)


def _phase2(nc, tc, dscr, xTs, xP, tP, tT, dP, probsT, ones_t, wcol, recip,
            p2_fp8, p3_fp8):
    """scoresT[s, q] = sum_k xT[k, s] tT[k, q]; probs-ish = exp(SCALE*(. + w_s));
    denominator colsums + DRAM-bounce transpose + reciprocal."""
    if p2_fp8:
        xP3 = [t.rearrange("p (i s) -> p i s", i=2) for t in xP]
        tP3 = [t.rearrange("p (i s) -> p i s", i=2) for t in tP]
    if p3_fp8:
        dP3 = [t.rearrange("p (i s) -> p i s", i=2) for t in dP]
        ones3 = ones_t.rearrange("p (i m) -> p i m", i=2)

    with (
        tc.tile_pool(name="ps2", bufs=4, space="PSUM") as ps2,
        tc.tile_pool(name="pcs", bufs=1, space="PSUM") as pcs,
        tc.tile_pool(name="etmp", bufs=4) as etmp,
    ):
        # fp8 path: colsum rows replicated across all 128 partitions (the
        # dual-fp8 Ldweights ISA check rejects 1-col-per-group stationaries)
        colsum = pcs.tile([P, S] if p3_fp8 else [1, S], F32,
                          name="colsum", tag="colsum")

        # n-outer so each n-chunk's denominator colsum can be emitted one
        # chunk behind the score chains (lagged so the PE never waits on the
        # exp/sub pipeline) — by the end of the score loop only the last
        # chunk's colsum remains, and the reciprocal is ready before phase
        # 3's first normalize needs it.
        srow2 = etmp.tile([1, S], F32, name="srow2", tag="srow2")

        def emit_colsum(n):
            if p3_fp8:
                for c in range(S_PAIR):
                    nc.tensor.matmul(
                        colsum[:, n * NT:(n + 1) * NT],
                        ones3,
                        dP3[c][:, :, n * NT:(n + 1) * NT],
                        start=(c == 0),
                        stop=(c == S_PAIR - 1),
                        perf_mode=DoubleRow,
                    )
                # denom chunk = colsum(d8) + S*K; per-chunk so the psum
                # pool's last consumer finishes right after the last colsum
                nc.vector.tensor_scalar_add(
                    srow2[0:1, n * NT:(n + 1) * NT],
                    colsum[0:1, n * NT:(n + 1) * NT], float(S) * K_EZ)
            else:
                for sm in range(S_T):
                    nc.tensor.matmul(
                        colsum[0:1, n * NT:(n + 1) * NT],
                        ones_t,
                        probsT[sm][:, n * NT:(n + 1) * NT],
                        start=(sm == 0),
                        stop=(sm == S_T - 1),
                    )
                nc.vector.tensor_copy(
                    srow2[0:1, n * NT:(n + 1) * NT],
                    colsum[0:1, n * NT:(n + 1) * NT])

        for n in range(S_N):
            for sm in range(S_T):
                ps = ps2.tile([P, NT], F32, name="ps_sc", tag="ps2", bufs=4)
                if p2_fp8:
                    for c in range(D_PAIR):
                        nc.tensor.matmul(
                            ps,
                            xP3[c][:, :, sm * P:(sm + 1) * P],
                            tP3[c][:, :, n * NT:(n + 1) * NT],
                            start=(c == 0),
                            stop=(c == D_PAIR - 1),
                            perf_mode=DoubleRow,
                        )
                else:
                    for kc in range(D_T):
                        nc.tensor.matmul(
                            ps,
                            xTs[:, kc * S + sm * P: kc * S + (sm + 1) * P],
                            tT[kc][:, n * NT:(n + 1) * NT],
                            start=(kc == 0),
                            stop=(kc == D_T - 1),
                        )
                if p3_fp8:
                    et = etmp.tile([P, NT], BF16, name="et", tag="et", bufs=4)
                    nc.scalar.activation(out=et, in_=ps, func=Exp, scale=SCALE,
                                         bias=wcol[:, sm:sm + 1])
                    dst = dP[sm // 2][:, (sm % 2) * S + n * NT:
                                      (sm % 2) * S + (n + 1) * NT]
                    with nc.allow_low_precision("fp8 probs deviations by design"):
                        nc.vector.tensor_scalar_add(dst, et, -K_EZ)
                else:
                    nc.scalar.activation(
                        out=probsT[sm][:, n * NT:(n + 1) * NT], in_=ps, func=Exp,
                        scale=SCALE, bias=wcol[:, sm:sm + 1])
            if n >= 1:
                emit_colsum(n - 1)
        emit_colsum(S_N - 1)

        # denom -> transpose [1,S] -> [P,S_T] via DRAM bounce -> reciprocal
        dsum = dscr.tile([S], F32, name="dsum", tag="dsum")
        nc.sync.dma_start(out=dsum, in_=srow2)
        sums_pm = etmp.tile([P, S_T], F32, name="sums_pm", tag="sums_pm")
        nc.sync.dma_start(out=sums_pm, in_=dsum.rearrange("(m p) -> p m", p=P))
        nc.vector.reciprocal(recip, sums_pm)


def _phase3(nc, tc, dP, probsT, v8, v, kvs, recip, out_d, p3_fp8):
    """out[q, j] = (sum_s d8[s,q] v8[s,j] + K*vsum[j]) * recip[q]  (fp8), or
    out[q, j] = (sum_s probsT[s,q] v[s,j]) * recip[q]  (bf16)."""
    if p3_fp8:
        dP3 = [t.rearrange("p (i s) -> p i s", i=2) for t in dP]
        v83 = [t.rearrange("p (i s) -> p i s", i=2) for t in v8]
    with (
        tc.tile_pool(name="ps3", bufs=3, space="PSUM") as ps3,
        tc.tile_pool(name="outp", bufs=6) as outp,
    ):
        for qm in range(S_T):
            po = ps3.tile([P, DV], F32, name="po", tag="po", bufs=3)
            if p3_fp8:
                for c in range(S_PAIR):
                    st, sp = (c == 0), (c == S_PAIR - 1)
                    lhsT = dP3[c][:, :, qm * P:(qm + 1) * P]
                    for nv in range(DV_N):
                        nc.tensor.matmul(
                            po[:, nv * NT:(nv + 1) * NT],
                            lhsT,
                            v83[c][:, :, nv * NT:(nv + 1) * NT],
                            start=st,
                            stop=sp,
                            perf_mode=DoubleRow,
                        )
                for nv in range(DV_N):
                    t1 = outp.tile([P, NT], F32, name="t1", tag="t1", bufs=3)
                    nc.vector.tensor_add(
                        t1, po[:, nv * NT:(nv + 1) * NT],
                        kvs[:, nv * NT:(nv + 1) * NT])
                    o = outp.tile([P, NT], F32, name="o", tag="o", bufs=3)
                    nc.scalar.activation(out=o, in_=t1, func=Copy,
                                         scale=recip[:, qm:qm + 1])
                    nc.sync.dma_start(
                        out=out_d[qm * P:(qm + 1) * P, nv * NT:(nv + 1) * NT],
                        in_=o,
                    )
            else:
                for sc in range(S_T):
                    st, sp = (sc == 0), (sc == S_T - 1)
                    lhsT = probsT[sc][:, qm * P:(qm + 1) * P]
                    for nv in range(DV_N):
                        nc.tensor.matmul(
                            po[:, nv * NT:(nv + 1) * NT],
                            lhsT,
                            v[sc][:, nv * NT:(nv + 1) * NT],
                            start=st,
                            stop=sp,
                        )
                for nv in range(DV_N):
                    o = outp.tile([P, NT], F32, name="o", tag="o", bufs=3)
                    nc.vector.tensor_scalar_mul(
                        o, po[:, nv * NT:(nv + 1) * NT], recip[:, qm:qm + 1]
                    )
                    nc.sync.dma_start(
                        out=out_d[qm * P:(qm + 1) * P, nv * NT:(nv + 1) * NT],
                        in_=o,
                    )


_CACHED = {}


def _build(p2_fp8=P2_FP8, p3_fp8=P3_FP8):
    key = (p2_fp8, p3_fp8)
    if key not in _CACHED:
        nc = bacc.Bacc(
            "TRN2",
            target_bir_lowering=False,
            debug=False,
            num_devices=B,
        )
        _emit(nc, p2_fp8, p3_fp8)
        nc.compile()
        _CACHED[key] = nc
    return _CACHED[key]


def _prep_inputs(x, Wq, bq, Wk, bk, Wv, bv, p3_fp8):
    x = np.asarray(x, dtype=np.float32)
    Wq = np.asarray(Wq, dtype=np.float32)
    Wk = np.asarray(Wk, dtype=np.float32)
    Wv = np.asarray(Wv, dtype=np.float32)
    bq = np.asarray(bq, dtype=np.float32)
    bk = np.asarray(bk, dtype=np.float32)
    bv = np.asarray(bv, dtype=np.float32)

    bf = ml_dtypes.bfloat16
    M = Wq @ Wk.T  # [D, DK]; the bq-only/bk-only score terms drop in softmax
    c = Wk @ bq    # w = x . c: the per-s score bias, applied inside the exp
    Wm_b = np.ascontiguousarray(M.astype(bf))
    Wv_b = np.ascontiguousarray(Wv.astype(bf))
    biasv = np.ascontiguousarray(np.broadcast_to(bv, (P, DV)).astype(np.float32))

    in_maps = []
    for b in range(B):
        xb = x[b]
        w = SCALE * (xb.astype(bf).astype(np.float32) @ c)  # [S]
        m = {
            "xT": np.ascontiguousarray(xb.T.astype(bf)),
            "Wm": Wm_b,
            "Wv": Wv_b,
            "wcol": np.ascontiguousarray(w.reshape(S_T, P).T.astype(np.float32)),
            "biasv": biasv,
        }
        if p3_fp8:
            vsum = xb.sum(axis=0) @ Wv + S * bv  # f32 colsum of v
            m["kvsum"] = np.ascontiguousarray(
                np.broadcast_to(K_EZ * vsum, (P, DV)).astype(np.float32))
        in_maps.append(m)
    return in_maps


def kernel(x, Wq, bq, Wk, bk, Wv, bv):
    in_maps = _prep_inputs(x, Wq, bq, Wk, bk, Wv, bv, P3_FP8)
    nc = _build()
    res = bass_utils.run_bass_kernel_spmd(
        nc,
        in_maps,
        core_ids=list(range(B)),
        trace=bool(int(os.environ.get("KERNEL_TRACE", "0"))),
        tmpdir=os.environ.get("KERNEL_TRACE_DIR") or None,
    )
    kernel.last_result = res
    return np.stack([r["out"] for r in res.results], axis=0)


# revision 24
# speedup vs baseline: 1.0290x; 1.0290x over previous
"""Single-head attention (B=8, S=2048, d_model=dk=dv=1024) on 8 TRN2 NeuronCores.

Strategy: data-parallel over batch, one batch element per core, SPMD.

Algebraic reduction vs the naive formulation: since softmax over s is
invariant to per-row (q) offsets, scores = (x Wq + bq)(x Wk + bk)^T reduces to
scores_eff[i,j] = x_i (Wq Wk^T) x_j^T + x_j (Wk bq), dropping the k-projection
entirely. M = Wq Wk^T and c = Wk bq are precomputed on host; the w = x c term
enters as a per-partition bias in the exp activation. This removes one of the
three [S,D]x[D,D] projections.

fp8 acceleration (flag-controlled): the two S x S matmuls (scores, AV) run in
fp8e4m3 with DoubleRow perf mode (2 k-groups per instruction, 2x PE
throughput). For the AV matmul, probs are stored as d8 = fp8(exp(z) - K) for
a hardcoded constant K ~ E[exp(z)]: the quantization noise then scales with
the small deviation rather than the full magnitude. The exact correction
out = (d8 @ v8 + K * colsum(v)) / denom uses colsum(v) computed on host in
f32, replicated across partitions; denom = colsum(d8) + S*K comes from
ones-stationary DoubleRow matmuls plus a DRAM-bounce transpose.

Matmul accumulation is always fp32 in PSUM. Phase 1 (projections t = x M,
v = x Wv, w = x c) stays bf16.
"""

import os
import sys

import numpy as np

try:
    import concourse.bass as bass  # noqa: F401
except ImportError:
    sys.path.insert(0, "/opt/trn_rl_repo")

import ml_dtypes

import concourse.bass as bass
import concourse.tile as tile
from concourse import bacc, mybir
from concourse import bass_utils

BF16 = mybir.dt.bfloat16
F32 = mybir.dt.float32
FP8 = mybir.dt.float8e4

B = 8
S = 2048
D = 1024  # d_model
DK = 1024
DV = 1024
P = 128  # partitions
NT = 512  # matmul free-dim tile (one PSUM bank of fp32)

D_T = D // P      # 8   contraction tiles over d_model
DK_T = DK // P    # 8   partition tiles of tT
S_T = S // P      # 16  partition tiles of v / probsT / out
S_N = S // NT     # 4   free-dim chunks over S
DV_N = DV // NT   # 2   free-dim chunks over dv
D_PAIR = D_T // 2   # 4  fp8 DoubleRow pairs over d
S_PAIR = S_T // 2   # 8  fp8 DoubleRow pairs over s

SCALE = 1.0 / float(np.sqrt(np.float32(DK)))
K_EZ = 1.078  # ~E[exp(scale*scores)] for this input distribution

P2_FP8 = bool(int(os.environ.get("KERNEL_P2_FP8", "1")))
P3_FP8 = bool(int(os.environ.get("KERNEL_P3_FP8", "1")))

DoubleRow = mybir.MatmulPerfMode.DoubleRow
Exp = mybir.ActivationFunctionType.Exp
Copy = mybir.ActivationFunctionType.Copy


def _emit(nc, p2_fp8, p3_fp8):
    xT_d = nc.dram_tensor("xT", [D, S], BF16, kind="ExternalInput").ap()
    Wm_d = nc.dram_tensor("Wm", [D, DK], BF16, kind="ExternalInput").ap()
    Wv_d = nc.dram_tensor("Wv", [D, DV], BF16, kind="ExternalInput").ap()
    wcol_d = nc.dram_tensor("wcol", [P, S_T], F32, kind="ExternalInput").ap()
    biasv_d = nc.dram_tensor("biasv", [P, DV], F32, kind="ExternalInput").ap()
    if p3_fp8:
        kvs_d = nc.dram_tensor("kvsum", [P, DV], F32, kind="ExternalInput").ap()
    out_d = nc.dram_tensor("out", [S, DV], F32, kind="ExternalOutput").ap()

    with tile.TileContext(nc) as tc:
        with (
            tc.tile_pool(name="persist", bufs=1) as persist,
            tc.tile_pool(name="dscr", bufs=1, space="DRAM") as dscr,
        ):
            # x^T stays resident: phase 1 contracts over it, phase 2 uses it
            # (or its fp8 copy) as the stationary side of the scores matmul.
            xTs = persist.tile([P, D_T * S], BF16, name="xTs", tag="xTs")
            if p2_fp8:
                xP = [persist.tile([P, 2 * S], FP8, name=f"xP{c}", tag=f"xP{c}")
                      for c in range(D_PAIR)]
                tP = [persist.tile([P, 2 * S], FP8, name=f"tP{c}", tag=f"tP{c}")
                      for c in range(D_PAIR)]
                tT = None
            else:
                xP = tP = None
                tT = [persist.tile([P, S], BF16, name=f"tT{i}", tag=f"tT{i}")
                      for i in range(DK_T)]
            if p3_fp8:
                v8 = [persist.tile([P, 2 * DV], FP8, name=f"v8{c}", tag=f"v8{c}")
                      for c in range(S_PAIR)]
                v = None
                kvs = persist.tile([P, DV], F32, name="kvs", tag="kvs")
                # [128, 2, 128] all-ones stationary for the denominator
                # colsums (1-col-per-group stationaries fail the dual-fp8
                # Ldweights ISA check, so the colsum output is replicated
                # across all 128 partitions instead).
                ones8 = persist.tile([P, 2 * P], FP8, name="ones8", tag="ones8")
                nc.vector.memset(ones8, 1.0)
            else:
                v8 = kvs = ones8 = None
                v = [persist.tile([P, DV], BF16, name=f"v{i}", tag=f"v{i}")
                     for i in range(S_T)]
                ones = persist.tile([P, 1], BF16, name="ones", tag="ones")
                nc.vector.memset(ones, 1.0)
            wcol = persist.tile([P, S_T], F32, name="wcol", tag="wcol")
            recip = persist.tile([P, S_T], F32, name="recip", tag="recip")

            _phase1(nc, tc, xT_d, Wm_d, Wv_d, wcol_d, biasv_d,
                    kvs_d if p3_fp8 else None, xTs, xP, tP, tT, v8, v, kvs,
                    wcol, p2_fp8, p3_fp8)

            with tc.tile_pool(name="probs", bufs=1) as probs_pool:
                if p3_fp8:
                    dP = [probs_pool.tile([P, 2 * S], FP8, name=f"dP{c}",
                                          tag=f"dP{c}") for c in range(S_PAIR)]
                    probsT = None
                else:
                    dP = None
                    probsT = [probs_pool.tile([P, S], BF16, name=f"pT{i}",
                                              tag=f"pT{i}") for i in range(S_T)]
                _phase2(nc, tc, dscr, xTs, xP, tP, tT, dP, probsT,
                        ones8 if p3_fp8 else ones, wcol, recip, p2_fp8, p3_fp8)
                _phase3(nc, tc, dP, probsT, v8, v, kvs, recip, out_d, p3_fp8)


def _phase1(nc, tc, xT_d, Wm_d, Wv_d, wcol_d, biasv_d, kvs_d, xTs, xP,
            tP, tT, v8, v, kvs, wcol, p2_fp8, p3_fp8):
    """tT = M^T x^T, v = x Wv + bv (w comes precomputed from host)."""
    with tc.tile_pool(name="inp", bufs=1) as inp:
        Wms = inp.tile([P, D_T * DK], BF16, name="Wms", tag="Wms")
        Wvs = inp.tile([P, D_T * DV], BF16, name="Wvs", tag="Wvs")
        biasv = inp.tile([P, DV], F32, name="biasv", tag="biasv")

        # Warmup matmuls on memset tiles: the PE pstate ramp (0.65 -> 1.2 ->
        # 2.4 GHz over ~3us of continuous execution) burns off during the
        # DMA lead-in instead of inflating the first real chains.
        wst = inp.tile([P, P], BF16, name="wst", tag="wst")
        wmv = inp.tile([P, NT], BF16, name="wmv", tag="wmv")
        nc.vector.memset(wst, 0.0)
        nc.vector.memset(wmv, 0.0)
        with tc.tile_pool(name="pwarm", bufs=2, space="PSUM") as pwarm:
            for i in range(14):
                wps = pwarm.tile([P, NT], F32, name="wps", tag="wps", bufs=2)
                nc.tensor.matmul(wps, wst, wmv, start=True, stop=True)

        xT3 = xTs.rearrange("p (c s) -> p c s", c=D_T)
        Wm3 = Wms.rearrange("p (c k) -> p c k", c=D_T)
        xTd3 = xT_d.rearrange("(c p) s -> p c s", p=P)
        Wmd3 = Wm_d.rearrange("(c p) k -> p c k", p=P)
        Wvd3 = Wv_d.rearrange("(c p) k -> p c k", p=P)

        # DMA order = consumption order. The first accumulation chain needs
        # Wm's m=0 block plus xT's n=0 chunk (kc-ascending), so issue those
        # first at fine granularity to cut the lead-in stall.
        nc.sync.dma_start(out=Wm3[:, :, 0:P], in_=Wmd3[:, :, 0:P])
        nc.sync.dma_start(out=xT3[:, :, 0:NT], in_=xTd3[:, :, 0:NT])
        for m in range(1, DK_T):
            nc.sync.dma_start(
                out=Wm3[:, :, m * P:(m + 1) * P], in_=Wmd3[:, :, m * P:(m + 1) * P]
            )
        for n in range(1, S_N):
            nc.sync.dma_start(
                out=xT3[:, :, n * NT:(n + 1) * NT], in_=xTd3[:, :, n * NT:(n + 1) * NT]
            )
        nc.sync.dma_start(out=Wvs, in_=Wvd3)
        nc.sync.dma_start(out=wcol, in_=wcol_d)
        nc.sync.dma_start(out=biasv, in_=biasv_d)
        if kvs_d is not None:
            nc.sync.dma_start(out=kvs, in_=kvs_d)

        def Wm_sl(kc, m):
            return Wms[:, kc * DK + m * P: kc * DK + (m + 1) * P]

        def xT_sl(kc, lo, hi):
            return xTs[:, kc * S + lo: kc * S + hi]

        with tc.tile_pool(name="ps1t", bufs=4, space="PSUM") as ps1t, \
             tc.tile_pool(name="ps1v", bufs=4, space="PSUM") as ps1v:
            # tT[m*P+p, i] = sum_d M[d, m*P+p] * xT[d, i]
            for n in range(S_N):
                # fp8 copy of x^T for the DoubleRow scores matmul, emitted
                # before the chain-gated last-n copies so the DVE does them
                # while waiting on the n=3 chains rather than at phase-2 start
                if p2_fp8 and n == S_N - 1:
                    for kc in range(D_T):
                        with nc.allow_low_precision("fp8 scores by design"):
                            nc.vector.tensor_copy(
                                xP[kc // 2][:, (kc % 2) * S:(kc % 2 + 1) * S],
                                xT_sl(kc, 0, S),
                            )
                for m in range(DK_T):
                    ps = ps1t.tile([P, NT], F32, name="ps_t", tag="ps1t", bufs=4)
                    for kc in range(D_T):
                        nc.tensor.matmul(
                            ps,
                            Wm_sl(kc, m),
                            xT_sl(kc, n * NT, (n + 1) * NT),
                            start=(kc == 0),
                            stop=(kc == D_T - 1),
                        )
                    if p2_fp8:
                        dst = tP[m // 2][:, (m % 2) * S + n * NT:
                                         (m % 2) * S + (n + 1) * NT]
                        with nc.allow_low_precision("fp8 scores by design"):
                            nc.vector.tensor_copy(dst, ps)
                    else:
                        nc.vector.tensor_copy(tT[m][:, n * NT:(n + 1) * NT], ps)

            # v[m*P+p, j] = sum_d xT[d, m*P+p] * Wv[d, j] + bv
            for m in range(S_T):
                for nv in range(DV_N):
                    ps = ps1v.tile([P, NT], F32, name="ps_v", tag="ps1v", bufs=4)
                    for kc in range(D_T):
                        nc.tensor.matmul(
                            ps,
                            xT_sl(kc, m * P, (m + 1) * P),
                            Wvs[:, kc * DV + nv * NT: kc * DV + (nv + 1) * NT],
                            start=(kc == 0),
                            stop=(kc == D_T - 1),
                        )
                    bsl = biasv[:, nv * NT:(nv + 1) * NT]
                    if p3_fp8:
                        dst = v8[m // 2][:, (m % 2) * DV + nv * NT:
                                         (m % 2) * DV + (nv + 1) * NT]
                        with nc.allow_low_precision("fp8 AV by design"):
                            nc.vector.tensor_add(dst, ps, bsl)
                    else:
                        nc.vector.tensor_add(v[m][:, nv * NT:(nv + 1) * NT], ps, bsl)


def _phase2(nc, tc, dscr, xTs, xP, tP, tT, dP, probsT, ones_t, wcol, recip,
            p2_fp8, p3_fp8):
    """scoresT[s, q] = sum_k xT[k, s] tT[k, q]; probs-ish = exp(SCALE*(. + w_s));
    denominator colsums + DRAM-bounce transpose + reciprocal."""
    if p2_fp8:
        xP3 = [t.rearrange("p (i s) -> p i s", i=2) for t in xP]
        tP3 = [t.rearrange("p (i s) -> p i s", i=2) for t in tP]
    if p3_fp8:
        dP3 = [t.rearrange("p (i s) -> p i s", i=2) for t in dP]
        ones3 = ones_t.rearrange("p (i m) -> p i m", i=2)

    with (
        tc.tile_pool(name="ps2", bufs=4, space="PSUM") as ps2,
        tc.tile_pool(name="pcs", bufs=1, space="PSUM") as pcs,
        tc.tile_pool(name="etmp", bufs=4) as etmp,
    ):
        # fp8 path: colsum rows replicated across all 128 partitions (the
        # dual-fp8 Ldweights ISA check rejects 1-col-per-group stationaries)
        colsum = pcs.tile([P, S] if p3_fp8 else [1, S], F32,
                          name="colsum", tag="colsum")

        # n-outer so each n-chunk's denominator colsum can be emitted one
        # chunk behind the score chains (lagged so the PE never waits on the
        # exp/sub pipeline) — by the end of the score loop only the last
        # chunk's colsum remains, and the reciprocal is ready before phase
        # 3's first normalize needs it.
        srow2 = etmp.tile([1, S], F32, name="srow2", tag="srow2")

        def emit_colsum(n):
            if p3_fp8:
                for c in range(S_PAIR):
                    nc.tensor.matmul(
                        colsum[:, n * NT:(n + 1) * NT],
                        ones3,
                        dP3[c][:, :, n * NT:(n + 1) * NT],
                        start=(c == 0),
                        stop=(c == S_PAIR - 1),
                        perf_mode=DoubleRow,
                    )
                # denom chunk = colsum(d8) + S*K; per-chunk so the psum
                # pool's last consumer finishes right after the last colsum
                nc.vector.tensor_scalar_add(
                    srow2[0:1, n * NT:(n + 1) * NT],
                    colsum[0:1, n * NT:(n + 1) * NT], float(S) * K_EZ)
            else:
                for sm in range(S_T):
                    nc.tensor.matmul(
                        colsum[0:1, n * NT:(n + 1) * NT],
                        ones_t,
                        probsT[sm][:, n * NT:(n + 1) * NT],
                        start=(sm == 0),
                        stop=(sm == S_T - 1),
                    )
                nc.vector.tensor_copy(
                    srow2[0:1, n * NT:(n + 1) * NT],
                    colsum[0:1, n * NT:(n + 1) * NT])

        for n in range(S_N):
            for sm in range(S_T):
                ps = ps2.tile([P, NT], F32, name="ps_sc", tag="ps2", bufs=4)
                if p2_fp8:
                    for c in range(D_PAIR):
                        nc.tensor.matmul(
                            ps,
                            xP3[c][:, :, sm * P:(sm + 1) * P],
                            tP3[c][:, :, n * NT:(n + 1) * NT],
                            start=(c == 0),
                            stop=(c == D_PAIR - 1),
                            perf_mode=DoubleRow,
                        )
                else:
                    for kc in range(D_T):
                        nc.tensor.matmul(
                            ps,
                            xTs[:, kc * S + sm * P: kc * S + (sm + 1) * P],
                            tT[kc][:, n * NT:(n + 1) * NT],
                            start=(kc == 0),
                            stop=(kc == D_T - 1),
                        )
                if p3_fp8:
                    et = etmp.tile([P, NT], BF16, name="et", tag="et", bufs=4)
                    nc.scalar.activation(out=et, in_=ps, func=Exp, scale=SCALE,
                                         bias=wcol[:, sm:sm + 1])
                    dst = dP[sm // 2][:, (sm % 2) * S + n * NT:
                                      (sm % 2) * S + (n + 1) * NT]
                    with nc.allow_low_precision("fp8 probs deviations by design"):
                        nc.vector.tensor_scalar_add(dst, et, -K_EZ)
                else:
                    nc.scalar.activation(
                        out=probsT[sm][:, n * NT:(n + 1) * NT], in_=ps, func=Exp,
                        scale=SCALE, bias=wcol[:, sm:sm + 1])
            if n >= 1:
                emit_colsum(n - 1)
        emit_colsum(S_N - 1)

        # denom -> transpose [1,S] -> [P,S_T] via DRAM bounce -> reciprocal
        dsum = dscr.tile([S], F32, name="dsum", tag="dsum")
        nc.sync.dma_start(out=dsum, in_=srow2)
        sums_pm = etmp.tile([P, S_T], F32, name="sums_pm", tag="sums_pm")
        nc.sync.dma_start(out=sums_pm, in_=dsum.rearrange("(m p) -> p m", p=P))
        nc.vector.reciprocal(recip, sums_pm)


def _phase3(nc, tc, dP, probsT, v8, v, kvs, recip, out_d, p3_fp8):
    """out[q, j] = (sum_s d8[s,q] v8[s,j] + K*vsum[j]) * recip[q]  (fp8), or
    out[q, j] = (sum_s probsT[s,q] v[s,j]) * recip[q]  (bf16)."""
    if p3_fp8:
        dP3 = [t.rearrange("p (i s) -> p i s", i=2) for t in dP]
        v83 = [t.rearrange("p (i s) -> p i s", i=2) for t in v8]
    with (
        tc.tile_pool(name="ps3", bufs=3, space="PSUM") as ps3,
        tc.tile_pool(name="outp", bufs=6) as outp,
    ):
        for qm in range(S_T):
            po = ps3.tile([P, DV], F32, name="po", tag="po", bufs=3)
            if p3_fp8:
                for c in range(S_PAIR):
                    st, sp = (c == 0), (c == S_PAIR - 1)
                    lhsT = dP3[c][:, :, qm * P:(qm + 1) * P]
                    for nv in range(DV_N):
                        nc.tensor.matmul(
                            po[:, nv * NT:(nv + 1) * NT],
                            lhsT,
                            v83[c][:, :, nv * NT:(nv + 1) * NT],
                            start=st,
                            stop=sp,
                            perf_mode=DoubleRow,
                        )
                for nv in range(DV_N):
                    t1 = outp.tile([P, NT], F32, name="t1", tag="t1", bufs=3)
                    nc.vector.tensor_add(
                        t1, po[:, nv * NT:(nv + 1) * NT],
                        kvs[:, nv * NT:(nv + 1) * NT])
                    o = outp.tile([P, NT], F32, name="o", tag="o", bufs=3)
                    nc.scalar.activation(out=o, in_=t1, func=Copy,
                                         scale=recip[:, qm:qm + 1])
                    nc.sync.dma_start(
                        out=out_d[qm * P:(qm + 1) * P, nv * NT:(nv + 1) * NT],
                        in_=o,
                    )
            else:
                for sc in range(S_T):
                    st, sp = (sc == 0), (sc == S_T - 1)
                    lhsT = probsT[sc][:, qm * P:(qm + 1) * P]
                    for nv in range(DV_N):
                        nc.tensor.matmul(
                            po[:, nv * NT:(nv + 1) * NT],
                            lhsT,
                            v[sc][:, nv * NT:(nv + 1) * NT],
                            start=st,
                            stop=sp,
                        )
                for nv in range(DV_N):
                    o = outp.tile([P, NT], F32, name="o", tag="o", bufs=3)
                    nc.vector.tensor_scalar_mul(
                        o, po[:, nv * NT:(nv + 1) * NT], recip[:, qm:qm + 1]
                    )
                    nc.sync.dma_start(
                        out=out_d[qm * P:(qm + 1) * P, nv * NT:(nv + 1) * NT],
                        in_=o,
                    )


_CACHED = {}


def _build(p2_fp8=P2_FP8, p3_fp8=P3_FP8):
    key = (p2_fp8, p3_fp8)
    if key not in _CACHED:
        nc = bacc.Bacc(
            "TRN2",
            target_bir_lowering=False,
            debug=False,
            num_devices=B,
        )
        _emit(nc, p2_fp8, p3_fp8)
        nc.compile()
        _CACHED[key] = nc
    return _CACHED[key]


def _prep_inputs(x, Wq, bq, Wk, bk, Wv, bv, p3_fp8):
    x = np.asarray(x, dtype=np.float32)
    Wq = np.asarray(Wq, dtype=np.float32)
    Wk = np.asarray(Wk, dtype=np.float32)
    Wv = np.asarray(Wv, dtype=np.float32)
    bq = np.asarray(bq, dtype=np.float32)
    bk = np.asarray(bk, dtype=np.float32)
    bv = np.asarray(bv, dtype=np.float32)

    bf = ml_dtypes.bfloat16
    M = Wq @ Wk.T  # [D, DK]; the bq-only/bk-only score terms drop in softmax
    c = Wk @ bq    # w = x . c: the per-s score bias, applied inside the exp
    Wm_b = np.ascontiguousarray(M.astype(bf))
    Wv_b = np.ascontiguousarray(Wv.astype(bf))
    biasv = np.ascontiguousarray(np.broadcast_to(bv, (P, DV)).astype(np.float32))

    in_maps = []
    for b in range(B):
        xb = x[b]
        w = SCALE * (xb.astype(bf).astype(np.float32) @ c)  # [S]
        m = {
            "xT": np.ascontiguousarray(xb.T.astype(bf)),
            "Wm": Wm_b,
            "Wv": Wv_b,
            "wcol": np.ascontiguousarray(w.reshape(S_T, P).T.astype(np.float32)),
            "biasv": biasv,
        }
        if p3_fp8:
            vsum = xb.sum(axis=0) @ Wv + S * bv  # f32 colsum of v
            m["kvsum"] = np.ascontiguousarray(
                np.broadcast_to(K_EZ * vsum, (P, DV)).astype(np.float32))
        in_maps.append(m)
    return in_maps


def kernel(x, Wq, bq, Wk, bk, Wv, bv):
    in_maps = _prep_inputs(x, Wq, bq, Wk, bk, Wv, bv, P3_FP8)
    nc = _build()
    res = bass_utils.run_bass_kernel_spmd(
        nc,
        in_maps,
        core_ids=list(range(B)),
        trace=bool(int(os.environ.get("KERNEL_TRACE", "0"))),
        tmpdir=os.environ.get("KERNEL_TRACE_DIR") or None,
    )
    kernel.last_result = res
    return np.stack([r["out"] for r in res.results], axis=0)
